# revision 40
# baseline (speedup 1.0000x reference)
"""Trainium2 Bass kernel for nn_AttentiveTransformer (TabNet attentive transformer).

Computes, for full inputs (N=16384, NA=256, F=2048):
    x  = a @ W.T + b
    xn = batchnorm(x)  (training mode, batch stats over all N rows)
    m  = sparsemax_ascending_variant(xn * ps)
    new_ps = ps * (1.5 - m)

Key identities:
 * The reference "sparsemax" sorts ascending; its k_z condition is monotone in
   the index, so k_z = D-1 always holds for this data regime and
   tau = (sum(z)+1)/(D-1), m = relu(z - tau). No sort.
 * BN stats from Gram partials: S1[f] = colsum_a . W_f, S2[f] = diag(W G W^T),
   both linear in per-core contributions -> one tiny (16KB) AllReduce.
   var = S2/N - (S1/N)^2; the affine normalization is folded into the matmul:
   W' = W*s, bias t = bn_b - (S1/N)*s (b cancels).
 * Accuracy budget is 2e-2 (harness gate); single-pass bf16 arithmetic with
   fp32 PSUM accumulation lands ~7e-3 worst-case (validated in numpy against
   the reference), so all I/O is bf16 and matmuls are single-pass bf16:
   halves HBM traffic (the binding roofline) and tensor-engine work vs the
   hi/lo-split fp32-grade version.

Sharding: data-parallel over rows, 2048 rows/core on 8 cores; a single 16KB
AllReduce merges the BN stats.
"""

import os
import sys
import numpy as np

for _p in ("/opt/trn_rl_repo",):
    if _p not in sys.path:
        sys.path.insert(0, _p)

KVAR = os.environ.get("KVAR", "")             # debug variant flags
KSTATS = os.environ.get("KSTATS", "full")     # "full": replicated Gram; "ag": shard Gram + AllGather
NTGPS = int(os.environ.get("NTGPS", "7"))     # row-tiles whose nps-mult runs on gpsimd
NT_GPS_SET = set(range(1, 14, 2)[:NTGPS]) if NTGPS else set()
UTACT = int(os.environ.get("UTACT", "2"))     # every UTACT-th tile's (GAMMA-m) on ACT

N, NA, F = 16384, 256, 2048
NCORES = 8
NSH = N // NCORES            # 2048 rows per core
P = 128                      # partitions
RT = NSH // P                # 16 row-tiles per core
FCW = 512                    # feature chunk width (psum bank)
FC = F // FCW                # 4 feature chunks
FP = F // P                  # 16 (cols of the [128,16] stats layout)
NAUG = NA + 1                # 257: a with ones column (colsum rides the Gram)
GAMMA = 1.5
BN_EPS = 1e-5
INV_D1 = 1.0 / (F - 1.0)     # 1/2047

_CACHE = {}


def _build_bass():
    import concourse.mybir as mybir
    import concourse.tile as tile
    from concourse import bacc
    from concourse.bass import ts

    fp32 = mybir.dt.float32
    bf16 = mybir.dt.bfloat16
    Alu = mybir.AluOpType
    Act = mybir.ActivationFunctionType

    nc = bacc.Bacc(
        "TRN2",
        target_bir_lowering=False,
        debug=False,
        enable_asserts=False,
        num_devices=NCORES,
    )

    # I/O (per core), all bf16 except the tiny BN affine params. In "full"
    # stats mode ah_aug is the FULL batch (replicated): each core computes
    # the global BN stats locally from the full Gram — no collective. In
    # "ag" mode it is the shard and a 16KB AllGather merges S1/S2 partials.
    NROWS = N if KSTATS == "full" else NSH
    ah_aug = nc.dram_tensor("ah_aug", [NROWS, NAUG], bf16, kind="ExternalInput").ap()
    ahT = nc.dram_tensor("ahT", [NA, NSH], bf16, kind="ExternalInput").ap()
    whT = nc.dram_tensor("whT", [NA, F], bf16, kind="ExternalInput").ap()
    ps_in = nc.dram_tensor("ps_in", [NSH, F], bf16, kind="ExternalInput").ap()
    bnw16 = nc.dram_tensor("bnw16", [P, FP], fp32, kind="ExternalInput").ap()
    bnb16 = nc.dram_tensor("bnb16", [P, FP], fp32, kind="ExternalInput").ap()
    m_out = nc.dram_tensor("m_out", [NSH, F], bf16, kind="ExternalOutput").ap()
    nps_out = nc.dram_tensor("nps_out", [NSH, F], bf16, kind="ExternalOutput").ap()

    ps_t = ps_in.rearrange("(t p) f -> t p f", p=P)
    m_t = m_out.rearrange("(t p) f -> t p f", p=P)
    nps_t = nps_out.rearrange("(t p) f -> t p f", p=P)
    # full a in [jumbo, p, sub, col] form: row = j*512 + s*128 + p
    ah_j = ah_aug.rearrange("(j s p) c -> j p s c", p=P, s=4)

    with tile.TileContext(nc) as tc:
        with tc.tile_pool(name="res", bufs=1) as res, \
             tc.tile_pool(name="dram", bufs=1, space="DRAM") as dram:
            psb = tc.alloc_tile_pool(name="psb", bufs=1)
            pro = tc.alloc_tile_pool(name="pro", bufs=1)

            # ---------------- constants; ACT table warm ----------------
            ones_colb = pro.tile([P, 1], bf16)
            nc.vector.memset(ones_colb, 1.0)
            # [128, P] "two-row selector": rows 0-1 ones, rest zero. Used as
            # a full-partition lhsT for the bias/fold broadcast matmuls so
            # every LDWEIGHTS is 128-partition (ldw-opt compatible).
            ones2 = res.tile([P, P], bf16)
            nc.vector.memset(ones2, 0.0)
            nc.vector.memset(ones2[0:2, :], 1.0)
            warm = pro.tile([1, 1], fp32)
            nc.vector.memset(warm, 1.0)
            nc.scalar.activation(warm, warm, Act.Sqrt)
            nc.scalar.activation(warm, warm, Act.Relu)

            # ---------------- phase 1: full-batch Gram (single-pass bf16) ---
            # G_aug = ah^T @ ah_aug over ALL 16384 rows; column NA of ah_aug
            # is ones, so G_aug[:, NA] is the colsum of a. Computing the full
            # Gram on every core (+7.4MB DMA, ~25us PE) replaces the stats
            # AllReduce, whose all-core rendezvous cost ~55us of dead time.
            NJ = NROWS // 512                # jumbo chunks of 512 rows
            g0 = pro.tile([P, NAUG], bf16)   # G rows 0:128 (+colsum col)
            g1 = pro.tile([P, NAUG], bf16)   # G rows 128:256
            with tc.tile_pool(name="pro1", bufs=1, space="PSUM") as pp1, \
                 tc.tile_pool(name="abig", bufs=1) as abigp:
                pg0 = pp1.tile([P, NAUG], fp32)
                pg1 = pp1.tile([P, NAUG], fp32)
                for j in range(NJ):
                    hch = abigp.tile([P, 4, NAUG], bf16, name="hch", tag="hch", bufs=6)
                    nc.sync.dma_start(hch, ah_j[j])
                    for s in range(4):
                        first = j == 0 and s == 0
                        last = j == NJ - 1 and s == 3
                        ah_t = hch[:, s, :]
                        nc.tensor.matmul(pg0, ah_t[:, 0:P], ah_t, start=first, stop=last)
                        nc.tensor.matmul(pg1, ah_t[:, P:NA], ah_t, start=first, stop=last)
                nc.vector.tensor_copy(g0, pg0)
                nc.vector.tensor_copy(g1, pg1)

            # ---------------- resident loads (after gram chunks queued) ----
            ah0 = res.tile([P, NSH], bf16)
            nc.sync.dma_start(ah0, ahT[0:P, :])
            ah1 = res.tile([P, NSH], bf16)
            nc.sync.dma_start(ah1, ahT[P:NA, :])
            w0r = pro.tile([P, F], bf16)
            nc.sync.dma_start(w0r, whT[0:P, :])
            w1r = pro.tile([P, F], bf16)
            nc.sync.dma_start(w1r, whT[P:NA, :])
            bnw_c = pro.tile([P, FP], fp32)
            nc.sync.dma_start(bnw_c, bnw16)
            bnb_c = pro.tile([P, FP], fp32)
            nc.sync.dma_start(bnb_c, bnb16)

            # ---------------- ps prefetch: all 16 row-tiles, sync ring -----
            psts = []
            for rt in range(RT):
                pst = psb.tile([P, F], bf16, name=f"pst{rt}")
                nc.sync.dma_start(pst, ps_t[rt])
                psts.append(pst)

            # ---------------- phase 2: S1/S2 partials ----------------
            # H = G @ W^T (contraction over G rows, two 128-halves);
            # S2 = colsum(H .* W^T); S1 = colsum_a @ W^T.
            srow = pro.tile([1, 2 * F], fp32)   # cols 0:F = S1 partial, F:2F = S2
            with tc.tile_pool(name="pro2", bufs=1, space="PSUM") as pp2, \
                 tc.tile_pool(name="qtmp", bufs=2) as qtmp:
                for fc in range(FC):
                    fsl = ts(fc, FCW)
                    ph0 = pp2.tile([P, FCW], fp32, name="ph0", tag="ph0", bufs=2)
                    nc.tensor.matmul(ph0, g0[:, 0:P], w0r[:, fsl], start=True, stop=False)
                    nc.tensor.matmul(ph0, g1[:, 0:P], w1r[:, fsl], start=False, stop=True)
                    ph1 = pp2.tile([P, FCW], fp32, name="ph1", tag="ph1", bufs=2)
                    nc.tensor.matmul(ph1, g0[:, P:NA], w0r[:, fsl], start=True, stop=False)
                    nc.tensor.matmul(ph1, g1[:, P:NA], w1r[:, fsl], start=False, stop=True)
                    q0 = qtmp.tile([P, FCW], bf16, name="q0")
                    nc.vector.tensor_tensor(q0, ph0, w0r[:, fsl], Alu.mult)
                    q1 = qtmp.tile([P, FCW], bf16, name="q1")
                    nc.vector.tensor_tensor(q1, ph1, w1r[:, fsl], Alu.mult)
                    ps2 = pp2.tile([1, FCW], fp32, name="ps2", tag="ps2", bufs=2)
                    nc.tensor.matmul(ps2, ones_colb, q0, start=True, stop=False)
                    nc.tensor.matmul(ps2, ones_colb, q1, start=False, stop=True)
                    ps1 = pp2.tile([1, FCW], fp32, name="ps1", tag="ps1", bufs=2)
                    nc.tensor.matmul(ps1, g0[:, NA:NAUG], w0r[:, fsl], start=True, stop=False)
                    nc.tensor.matmul(ps1, g1[:, NA:NAUG], w1r[:, fsl], start=False, stop=True)
                    nc.scalar.copy(srow[0:1, fsl], ps1)
                    nc.scalar.copy(srow[0:1, ts(FC + fc, FCW)], ps2)

            # ---------------- phase 3/4: merge partials; stats math ---------
            # (DRAM bounce: SBUF->SBUF DMA cannot scatter a row across
            # partitions, but DRAM APs are linear so the reload can.)
            sdr = dram.tile([1, 2 * F], fp32)
            nc.scalar.dma_start(sdr, srow)
            ssl = pro.tile([P, F], bf16)        # row 0: s hi, row 1: s lo
            nc.vector.memset(ssl, 0.0)
            ttl2 = res.tile([P, F], bf16)       # row 0: t hi, row 1: t lo
            nc.vector.memset(ttl2, 0.0)
            with tc.tile_pool(name="smath", bufs=1) as sm:
                st1 = sm.tile([P, FP], fp32)
                st2 = sm.tile([P, FP], fp32)
                if KSTATS == "ag":
                    cc_out = dram.tile([NCORES, 2 * F], fp32, addr_space="Shared")
                    nc.gpsimd.collective_compute(
                        "AllGather",
                        Alu.bypass,
                        replica_groups=[list(range(NCORES))],
                        ins=[sdr.opt()],
                        outs=[cc_out.opt()],
                    )
                    # ranks land r-major in the free dim: [p, r, c]
                    cc_r = cc_out.rearrange("r (two p c) -> two p r c", two=2, p=P)
                    for blk, stt in ((0, st1), (1, st2)):
                        sa = sm.tile([P, NCORES, FP], fp32, name=f"sa{blk}")
                        (nc.scalar if blk == 0 else nc.sync).dma_start(sa, cc_r[blk])
                        h1 = sm.tile([P, 4, FP], fp32, name=f"h1{blk}")
                        nc.vector.tensor_tensor(h1, sa[:, 0:4, :], sa[:, 4:8, :], Alu.add)
                        h2 = sm.tile([P, 2, FP], fp32, name=f"h2{blk}")
                        nc.vector.tensor_tensor(h2, h1[:, 0:2, :], h1[:, 2:4, :], Alu.add)
                        nc.vector.tensor_tensor(stt, h2[:, 0:1, :], h2[:, 1:2, :], Alu.add)
                else:
                    srow_r = sdr.rearrange("o (two p c) -> two (o p) c", two=2, p=P)
                    nc.scalar.dma_start(st1, srow_r[0])
                    nc.scalar.dma_start(st2, srow_r[1])
                sq = sm.tile([P, FP], fp32)
                nc.vector.tensor_tensor(sq, st1, st1, Alu.mult)
                # vv = S2 - S1^2/N + N*eps  (= N*(var+eps))
                vv = sm.tile([P, FP], fp32)
                nc.vector.scalar_tensor_tensor(vv, sq, -1.0 / N, st2, Alu.mult, Alu.add)
                nc.vector.tensor_scalar_add(vv, vv, float(N * BN_EPS))
                rr = sm.tile([P, FP], fp32)
                nc.scalar.activation(rr, vv, Act.Sqrt)
                y = sm.tile([P, FP], fp32)
                nc.vector.reciprocal(y, rr)
                # one Newton iteration for rsqrt (ACT Sqrt seed is coarse)
                yy = sm.tile([P, FP], fp32)
                nc.vector.tensor_tensor(yy, y, y, Alu.mult)
                vyy = sm.tile([P, FP], fp32)
                nc.vector.tensor_tensor(vyy, vv, yy, Alu.mult)
                w_ = sm.tile([P, FP], fp32)
                nc.vector.tensor_scalar(w_, vyy, -0.5, 1.5, Alu.mult, Alu.add)
                y2 = sm.tile([P, FP], fp32)
                nc.vector.tensor_tensor(y2, y, w_, Alu.mult)
                # s = sqrt(N) * y * bn_w; t = bn_b - (S1/N)*s  (b cancels)
                s_c = sm.tile([P, FP], fp32)
                nc.vector.scalar_tensor_tensor(s_c, y2, float(np.sqrt(N)), bnw_c, Alu.mult, Alu.mult)
                tm = sm.tile([P, FP], fp32)
                nc.vector.scalar_tensor_tensor(tm, st1, -1.0 / N, s_c, Alu.mult, Alu.mult)
                t_c = sm.tile([P, FP], fp32)
                nc.vector.tensor_tensor(t_c, tm, bnb_c, Alu.add)
                # bf16 hi/lo of s and t in the [128,16] layout, then SBUF->SBUF
                # DMAs gather them into row tiles (f = p*16 + c ordering)
                sh_c = sm.tile([P, FP], bf16)
                nc.vector.tensor_copy(sh_c, s_c)
                sl_c = sm.tile([P, FP], bf16)
                nc.vector.tensor_tensor(sl_c, s_c, sh_c, Alu.subtract)
                th_c = sm.tile([P, FP], bf16)
                nc.vector.tensor_copy(th_c, t_c)
                tl_c = sm.tile([P, FP], bf16)
                nc.vector.tensor_tensor(tl_c, t_c, th_c, Alu.subtract)
                nc.scalar.dma_start(ssl[0:1, :], sh_c)
                nc.gpsimd.dma_start(ssl[1:2, :], sl_c)
                nc.scalar.dma_start(ttl2[0:1, :], th_c)
                nc.gpsimd.dma_start(ttl2[1:2, :], tl_c)

            # ---------------- phase 5: fold scale into W^T ----------------
            w0p = res.tile([P, F], bf16)
            w1p = res.tile([P, F], bf16)
            with tc.tile_pool(name="pro3", bufs=2, space="PSUM") as pp3:
                for fc in range(FC):
                    fsl = ts(fc, FCW)
                    pb = pp3.tile([P, FCW], fp32, name="pb")
                    nc.tensor.matmul(pb, ones2, ssl[:, fsl], start=True, stop=True)
                    nc.vector.tensor_tensor(w0p[:, fsl], w0r[:, fsl], pb, Alu.mult)
                    nc.vector.tensor_tensor(w1p[:, fsl], w1r[:, fsl], pb, Alu.mult)
            pro.release()

            # ---------------- main loop over 16 row-tiles ----------------
            with tc.tile_pool(name="mx", bufs=8, space="PSUM") as mxp, \
                 tc.tile_pool(name="zb", bufs=3) as zb, \
                 tc.tile_pool(name="mb", bufs=3) as mb, \
                 tc.tile_pool(name="ub", bufs=3) as ub, \
                 tc.tile_pool(name="nb", bufs=3) as nb, \
                 tc.tile_pool(name="rsb", bufs=6) as rsb:
                mts = {}

                def epilogue(rt):
                    # ut = GAMMA - m (bf16 4x), nt = ut * ps; emitted one tile
                    # late so the DVE never stalls waiting on ACT's relu.
                    mt = mts.pop(rt)
                    ut = ub.tile([P, F], bf16, name="ut")
                    if UTACT and rt % UTACT == 0:
                        nc.scalar.activation(ut, mt, Act.Copy, bias=GAMMA, scale=-1.0)
                    else:
                        nc.vector.tensor_scalar(ut, mt, -1.0, GAMMA, Alu.mult, Alu.add)
                    nt = nb.tile([P, F], bf16, name="nt")
                    if rt in NT_GPS_SET:
                        nc.gpsimd.tensor_tensor(nt, ut, psts[rt], Alu.mult)
                        nc.gpsimd.dma_start(nps_t[rt], nt)
                    else:
                        nc.vector.tensor_tensor(nt, ut, psts[rt], Alu.mult)
                        nc.scalar.dma_start(nps_t[rt], nt)

                for rt in range(RT):
                    rsl = ts(rt, P)
                    pst = psts[rt]
                    px = mxp.tile([P, F], fp32, name="px", tag="px", bufs=2)
                    # pass-type-major: each lhsT loads once, streams 4 chunks
                    ptypes = [(ah0[:, rsl], w0p), (ah1[:, rsl], w1p),
                              (ones2, ttl2)]
                    for pi, (lhsT, rhs) in enumerate(ptypes):
                        for fc in range(FC):
                            nc.tensor.matmul(px[:, ts(fc, FCW)], lhsT, rhs[:, ts(fc, FCW)],
                                             start=(pi == 0), stop=(pi == len(ptypes) - 1))
                    # z = xn * ps over the row-tile; rs = rowsum(z)
                    zt = zb.tile([P, F], fp32, name="zt")
                    rs = rsb.tile([P, 1], fp32, name="rs")
                    nc.vector.scalar_tensor_tensor(
                        zt, px, 1.0, pst, Alu.mult, Alu.mult, accum_out=rs,
                    )
                    # tau = (sum(z)+1)/2047 = (rs+1)/2047
                    ntau = rsb.tile([P, 1], fp32, name="ntau")      # -tau
                    nc.vector.tensor_scalar(ntau, rs, -INV_D1, -INV_D1, Alu.mult, Alu.add)
                    # m = relu(z - tau) = relu(z + ntau)
                    mt = mb.tile([P, F], bf16, name="mt")
                    nc.scalar.activation(mt, zt, Act.Relu, bias=ntau, scale=1.0)
                    nc.scalar.dma_start(m_t[rt], mt)
                    mts[rt] = mt
                    if rt > 0:
                        epilogue(rt - 1)
                epilogue(RT - 1)
            psb.release()

    nc.compile()
    return nc


def _get_nc():
    if "nc" not in _CACHE:
        _CACHE["nc"] = _build_bass()
    return _CACHE["nc"]


def _make_in_maps(a, ps, W, b, bn_w, bn_b):
    import ml_dtypes

    bf = ml_dtypes.bfloat16
    ah = np.ascontiguousarray(a, dtype=np.float32).astype(bf)
    ps16 = np.ascontiguousarray(ps, dtype=np.float32).astype(bf)
    whT_np = np.ascontiguousarray(W.astype(np.float32).T.astype(bf))
    bnw16 = np.ascontiguousarray(bn_w.astype(np.float32).reshape(P, FP))
    bnb16 = np.ascontiguousarray(bn_b.astype(np.float32).reshape(P, FP))
    in_maps = []
    ah_aug = np.ascontiguousarray(
        np.concatenate([ah, np.ones((N, 1), bf)], axis=1))
    for c in range(NCORES):
        rows = slice(c * NSH, (c + 1) * NSH)
        ah_c = ah[rows]
        in_maps.append({
            "ah_aug": ah_aug if KSTATS == "full"
            else np.ascontiguousarray(ah_aug[rows]),
            "ahT": np.ascontiguousarray(ah_c.T),
            "whT": whT_np,
            "ps_in": np.ascontiguousarray(ps16[rows]),
            "bnw16": bnw16,
            "bnb16": bnb16,
        })
    return in_maps


def _maybe_patch_ldwopt():
    """Optionally flip walrus's --enable-ldw-opt (default false in bass_utils)."""
    if os.environ.get("BASS_LDW_OPT") != "1":
        return
    from concourse import bass_utils as bu
    if getattr(bu, "_ldwopt_patched", False):
        return
    orig = bu.run_command

    def patched(argv, **kw):
        argv = [x.replace("--enable-ldw-opt=false", "--enable-ldw-opt=true")
                if isinstance(x, str) else x for x in argv]
        return orig(argv, **kw)

    bu.run_command = patched
    bu._ldwopt_patched = True


def run(a, ps, W, b, bn_w, bn_b, trace=False, **kw):
    """Run the kernel on the 8 NeuronCores; returns ((m, new_ps), BassKernelResults)."""
    from concourse import bass_utils

    _maybe_patch_ldwopt()
    nc = _get_nc()
    in_maps = _make_in_maps(a, ps, W, b, bn_w, bn_b)
    res = bass_utils.run_bass_kernel_spmd(
        nc, in_maps, core_ids=list(range(NCORES)), trace=trace, **kw,
    )
    m = np.concatenate(
        [np.asarray(r["m_out"]).astype(np.float32) for r in res.results], axis=0)
    nps = np.concatenate(
        [np.asarray(r["nps_out"]).astype(np.float32) for r in res.results], axis=0)
    return (m, nps), res


def kernel(a, ps, W, b, bn_w, bn_b):
    (m, nps), _ = run(a, ps, W, b, bn_w, bn_b, trace=False)
    return m, nps


if __name__ == "__main__":
    rng = np.random.default_rng(0)
    a = rng.standard_normal((N, NA), dtype=np.float32)
    ps = rng.random((N, F), dtype=np.float32)
    lim = 1.0 / np.sqrt(NA)
    W = rng.uniform(-lim, lim, (F, NA)).astype(np.float32)
    b = rng.uniform(-lim, lim, (F,)).astype(np.float32)
    bn_w = np.ones((F,), np.float32)
    bn_b = np.zeros((F,), np.float32)
    (m, nps), res = run(a, ps, W, b, bn_w, bn_b)
    print("m", m.shape, m.dtype, "nps", nps.shape)
    print("exec_time_ns:", res.exec_time_ns)


# revision 41
# speedup vs baseline: 1.1429x; 1.1429x over previous
"""Trainium2 Bass kernel for nn_AttentiveTransformer (TabNet attentive transformer).

Computes, for full inputs (N=16384, NA=256, F=2048):
    x  = a @ W.T + b
    xn = batchnorm(x)  (training mode, batch stats over all N rows)
    m  = sparsemax_ascending_variant(xn * ps)
    new_ps = ps * (1.5 - m)

Key identities:
 * The reference "sparsemax" sorts ascending; its k_z condition is monotone in
   the index, so k_z = D-1 always holds for this data regime and
   tau = (sum(z)+1)/(D-1), m = relu(z - tau). No sort.
 * BN stats from Gram partials: S1[f] = colsum_a . W_f, S2[f] = diag(W G W^T),
   both linear in per-core contributions -> one tiny (16KB) AllReduce.
   var = S2/N - (S1/N)^2; the affine normalization is folded into the matmul:
   W' = W*s, bias t = bn_b - (S1/N)*s (b cancels).
 * Accuracy budget is 2e-2 (harness gate); single-pass bf16 arithmetic with
   fp32 PSUM accumulation lands ~7e-3 worst-case (validated in numpy against
   the reference), so all I/O is bf16 and matmuls are single-pass bf16:
   halves HBM traffic (the binding roofline) and tensor-engine work vs the
   hi/lo-split fp32-grade version.

Sharding: data-parallel over rows, 2048 rows/core on 8 cores; a single 16KB
AllReduce merges the BN stats.
"""

import os
import sys
import numpy as np

for _p in ("/opt/trn_rl_repo",):
    if _p not in sys.path:
        sys.path.insert(0, _p)

KVAR = os.environ.get("KVAR", "")             # debug variant flags
KSTATS = os.environ.get("KSTATS", "full")     # "full": replicated Gram; "ag": shard Gram + AllGather
NTGPS = int(os.environ.get("NTGPS", "7"))     # row-tiles whose nps-mult runs on gpsimd
NT_GPS_SET = set(range(1, 14, 2)[:NTGPS]) if NTGPS else set()
UTACT = int(os.environ.get("UTACT", "0"))     # every UTACT-th tile's (GAMMA-m) on ACT

N, NA, F = 16384, 256, 2048
NCORES = 8
NSH = N // NCORES            # 2048 rows per core
P = 128                      # partitions
RT = NSH // P                # 16 row-tiles per core
FCW = 512                    # feature chunk width (psum bank)
FC = F // FCW                # 4 feature chunks
FP = F // P                  # 16 (cols of the [128,16] stats layout)
NAUG = NA + 1                # 257: a with ones column (colsum rides the Gram)
GAMMA = 1.5
BN_EPS = 1e-5
INV_D1 = 1.0 / (F - 1.0)     # 1/2047

_CACHE = {}


def _build_bass():
    import concourse.mybir as mybir
    import concourse.tile as tile
    from concourse import bacc
    from concourse.bass import ts

    fp32 = mybir.dt.float32
    bf16 = mybir.dt.bfloat16
    Alu = mybir.AluOpType
    Act = mybir.ActivationFunctionType

    nc = bacc.Bacc(
        "TRN2",
        target_bir_lowering=False,
        debug=False,
        enable_asserts=False,
        num_devices=NCORES,
    )

    # I/O (per core), all bf16 except the tiny BN affine params. In "full"
    # stats mode ah_aug is the FULL batch (replicated): each core computes
    # the global BN stats locally from the full Gram — no collective. In
    # "ag" mode it is the shard and a 16KB AllGather merges S1/S2 partials.
    NROWS = N if KSTATS == "full" else NSH
    ah_aug = nc.dram_tensor("ah_aug", [NROWS, NAUG], bf16, kind="ExternalInput").ap()
    ahT = nc.dram_tensor("ahT", [NA, NSH], bf16, kind="ExternalInput").ap()
    whT = nc.dram_tensor("whT", [NA, F], bf16, kind="ExternalInput").ap()
    ps_in = nc.dram_tensor("ps_in", [NSH, F], bf16, kind="ExternalInput").ap()
    bnw16 = nc.dram_tensor("bnw16", [P, FP], fp32, kind="ExternalInput").ap()
    bnb16 = nc.dram_tensor("bnb16", [P, FP], fp32, kind="ExternalInput").ap()
    m_out = nc.dram_tensor("m_out", [NSH, F], bf16, kind="ExternalOutput").ap()
    nps_out = nc.dram_tensor("nps_out", [NSH, F], bf16, kind="ExternalOutput").ap()

    ps_t = ps_in.rearrange("(t p) f -> t p f", p=P)
    m_t = m_out.rearrange("(t p) f -> t p f", p=P)
    nps_t = nps_out.rearrange("(t p) f -> t p f", p=P)
    # full a in [jumbo, p, sub, col] form: row = j*512 + s*128 + p
    ah_j = ah_aug.rearrange("(j s p) c -> j p s c", p=P, s=4)

    with tile.TileContext(nc) as tc:
        with tc.tile_pool(name="res", bufs=1) as res, \
             tc.tile_pool(name="dram", bufs=1, space="DRAM") as dram:
            psb = tc.alloc_tile_pool(name="psb", bufs=1)
            pro = tc.alloc_tile_pool(name="pro", bufs=1)

            # ---------------- constants; ACT table warm ----------------
            ones_colb = pro.tile([P, 1], bf16)
            nc.vector.memset(ones_colb, 1.0)
            # [128, P] "two-row selector": rows 0-1 ones, rest zero. Used as
            # a full-partition lhsT for the bias/fold broadcast matmuls so
            # every LDWEIGHTS is 128-partition (ldw-opt compatible).
            ones2 = res.tile([P, P], bf16)
            nc.vector.memset(ones2, 0.0)
            nc.vector.memset(ones2[0:2, :], 1.0)
            warm = pro.tile([1, 1], fp32)
            nc.vector.memset(warm, 1.0)
            nc.scalar.activation(warm, warm, Act.Sqrt)
            nc.scalar.activation(warm, warm, Act.Relu)

            # ---------------- phase 1: full-batch Gram (single-pass bf16) ---
            # G_aug = ah^T @ ah_aug over ALL 16384 rows; column NA of ah_aug
            # is ones, so G_aug[:, NA] is the colsum of a. Computing the full
            # Gram on every core (+7.4MB DMA, ~25us PE) replaces the stats
            # AllReduce, whose all-core rendezvous cost ~55us of dead time.
            NJ = NROWS // 512                # jumbo chunks of 512 rows
            g0 = pro.tile([P, NAUG], bf16)   # G rows 0:128 (+colsum col)
            g1 = pro.tile([P, NAUG], bf16)   # G rows 128:256
            with tc.tile_pool(name="pro1", bufs=1, space="PSUM") as pp1, \
                 tc.tile_pool(name="abig", bufs=1) as abigp:
                pg0 = pp1.tile([P, NAUG], fp32)
                pg1 = pp1.tile([P, NAUG], fp32)
                for j in range(NJ):
                    hch = abigp.tile([P, 4, NAUG], bf16, name="hch", tag="hch", bufs=6)
                    nc.sync.dma_start(hch, ah_j[j])
                    for s in range(4):
                        first = j == 0 and s == 0
                        last = j == NJ - 1 and s == 3
                        ah_t = hch[:, s, :]
                        nc.tensor.matmul(pg0, ah_t[:, 0:P], ah_t, start=first, stop=last)
                        nc.tensor.matmul(pg1, ah_t[:, P:NA], ah_t, start=first, stop=last)
                nc.vector.tensor_copy(g0, pg0)
                nc.vector.tensor_copy(g1, pg1)

            # ---------------- resident loads (after gram chunks queued) ----
            ah0 = res.tile([P, NSH], bf16)
            nc.sync.dma_start(ah0, ahT[0:P, :])
            ah1 = res.tile([P, NSH], bf16)
            nc.sync.dma_start(ah1, ahT[P:NA, :])
            w0r = pro.tile([P, F], bf16)
            nc.sync.dma_start(w0r, whT[0:P, :])
            w1r = pro.tile([P, F], bf16)
            nc.sync.dma_start(w1r, whT[P:NA, :])
            bnw_c = pro.tile([P, FP], fp32)
            nc.sync.dma_start(bnw_c, bnw16)
            bnb_c = pro.tile([P, FP], fp32)
            nc.sync.dma_start(bnb_c, bnb16)

            # ---------------- ps prefetch: all 16 row-tiles, sync ring -----
            psts = []
            for rt in range(RT):
                pst = psb.tile([P, F], bf16, name=f"pst{rt}")
                nc.sync.dma_start(pst, ps_t[rt])
                psts.append(pst)

            # ---------------- phase 2: S1/S2 partials ----------------
            # H = G @ W^T (contraction over G rows, two 128-halves);
            # S2 = colsum(H .* W^T); S1 = colsum_a @ W^T.
            srow = pro.tile([1, 2 * F], fp32)   # cols 0:F = S1 partial, F:2F = S2
            with tc.tile_pool(name="pro2", bufs=1, space="PSUM") as pp2, \
                 tc.tile_pool(name="qtmp", bufs=2) as qtmp:
                for fc in range(FC):
                    fsl = ts(fc, FCW)
                    ph0 = pp2.tile([P, FCW], fp32, name="ph0", tag="ph0", bufs=2)
                    nc.tensor.matmul(ph0, g0[:, 0:P], w0r[:, fsl], start=True, stop=False)
                    nc.tensor.matmul(ph0, g1[:, 0:P], w1r[:, fsl], start=False, stop=True)
                    ph1 = pp2.tile([P, FCW], fp32, name="ph1", tag="ph1", bufs=2)
                    nc.tensor.matmul(ph1, g0[:, P:NA], w0r[:, fsl], start=True, stop=False)
                    nc.tensor.matmul(ph1, g1[:, P:NA], w1r[:, fsl], start=False, stop=True)
                    q0 = qtmp.tile([P, FCW], bf16, name="q0")
                    nc.vector.tensor_tensor(q0, ph0, w0r[:, fsl], Alu.mult)
                    q1 = qtmp.tile([P, FCW], bf16, name="q1")
                    nc.vector.tensor_tensor(q1, ph1, w1r[:, fsl], Alu.mult)
                    ps2 = pp2.tile([1, FCW], fp32, name="ps2", tag="ps2", bufs=2)
                    nc.tensor.matmul(ps2, ones_colb, q0, start=True, stop=False)
                    nc.tensor.matmul(ps2, ones_colb, q1, start=False, stop=True)
                    ps1 = pp2.tile([1, FCW], fp32, name="ps1", tag="ps1", bufs=2)
                    nc.tensor.matmul(ps1, g0[:, NA:NAUG], w0r[:, fsl], start=True, stop=False)
                    nc.tensor.matmul(ps1, g1[:, NA:NAUG], w1r[:, fsl], start=False, stop=True)
                    nc.scalar.copy(srow[0:1, fsl], ps1)
                    nc.scalar.copy(srow[0:1, ts(FC + fc, FCW)], ps2)

            # ---------------- phase 3/4: merge partials; stats math ---------
            # (DRAM bounce: SBUF->SBUF DMA cannot scatter a row across
            # partitions, but DRAM APs are linear so the reload can.)
            sdr = dram.tile([1, 2 * F], fp32)
            nc.scalar.dma_start(sdr, srow)
            ssl = pro.tile([P, F], bf16)        # row 0: s hi, row 1: s lo
            nc.vector.memset(ssl, 0.0)
            ttl2 = res.tile([P, F], bf16)       # row 0: t hi, row 1: t lo
            nc.vector.memset(ttl2, 0.0)
            with tc.tile_pool(name="smath", bufs=1) as sm:
                st1 = sm.tile([P, FP], fp32)
                st2 = sm.tile([P, FP], fp32)
                if KSTATS == "ag":
                    cc_out = dram.tile([NCORES, 2 * F], fp32, addr_space="Shared")
                    nc.gpsimd.collective_compute(
                        "AllGather",
                        Alu.bypass,
                        replica_groups=[list(range(NCORES))],
                        ins=[sdr.opt()],
                        outs=[cc_out.opt()],
                    )
                    # ranks land r-major in the free dim: [p, r, c]
                    cc_r = cc_out.rearrange("r (two p c) -> two p r c", two=2, p=P)
                    for blk, stt in ((0, st1), (1, st2)):
                        sa = sm.tile([P, NCORES, FP], fp32, name=f"sa{blk}")
                        (nc.scalar if blk == 0 else nc.sync).dma_start(sa, cc_r[blk])
                        h1 = sm.tile([P, 4, FP], fp32, name=f"h1{blk}")
                        nc.vector.tensor_tensor(h1, sa[:, 0:4, :], sa[:, 4:8, :], Alu.add)
                        h2 = sm.tile([P, 2, FP], fp32, name=f"h2{blk}")
                        nc.vector.tensor_tensor(h2, h1[:, 0:2, :], h1[:, 2:4, :], Alu.add)
                        nc.vector.tensor_tensor(stt, h2[:, 0:1, :], h2[:, 1:2, :], Alu.add)
                else:
                    srow_r = sdr.rearrange("o (two p c) -> two (o p) c", two=2, p=P)
                    nc.scalar.dma_start(st1, srow_r[0])
                    nc.scalar.dma_start(st2, srow_r[1])
                sq = sm.tile([P, FP], fp32)
                nc.vector.tensor_tensor(sq, st1, st1, Alu.mult)
                # vv = S2 - S1^2/N + N*eps  (= N*(var+eps))
                vv = sm.tile([P, FP], fp32)
                nc.vector.scalar_tensor_tensor(vv, sq, -1.0 / N, st2, Alu.mult, Alu.add)
                nc.vector.tensor_scalar_add(vv, vv, float(N * BN_EPS))
                rr = sm.tile([P, FP], fp32)
                nc.scalar.activation(rr, vv, Act.Sqrt)
                y = sm.tile([P, FP], fp32)
                nc.vector.reciprocal(y, rr)
                # one Newton iteration for rsqrt (ACT Sqrt seed is coarse)
                yy = sm.tile([P, FP], fp32)
                nc.vector.tensor_tensor(yy, y, y, Alu.mult)
                vyy = sm.tile([P, FP], fp32)
                nc.vector.tensor_tensor(vyy, vv, yy, Alu.mult)
                w_ = sm.tile([P, FP], fp32)
                nc.vector.tensor_scalar(w_, vyy, -0.5, 1.5, Alu.mult, Alu.add)
                y2 = sm.tile([P, FP], fp32)
                nc.vector.tensor_tensor(y2, y, w_, Alu.mult)
                # s = sqrt(N) * y * bn_w; t = bn_b - (S1/N)*s  (b cancels)
                s_c = sm.tile([P, FP], fp32)
                nc.vector.scalar_tensor_tensor(s_c, y2, float(np.sqrt(N)), bnw_c, Alu.mult, Alu.mult)
                tm = sm.tile([P, FP], fp32)
                nc.vector.scalar_tensor_tensor(tm, st1, -1.0 / N, s_c, Alu.mult, Alu.mult)
                t_c = sm.tile([P, FP], fp32)
                nc.vector.tensor_tensor(t_c, tm, bnb_c, Alu.add)
                # bf16 hi/lo of s and t in the [128,16] layout, then SBUF->SBUF
                # DMAs gather them into row tiles (f = p*16 + c ordering)
                sh_c = sm.tile([P, FP], bf16)
                nc.vector.tensor_copy(sh_c, s_c)
                sl_c = sm.tile([P, FP], bf16)
                nc.vector.tensor_tensor(sl_c, s_c, sh_c, Alu.subtract)
                th_c = sm.tile([P, FP], bf16)
                nc.vector.tensor_copy(th_c, t_c)
                tl_c = sm.tile([P, FP], bf16)
                nc.vector.tensor_tensor(tl_c, t_c, th_c, Alu.subtract)
                nc.scalar.dma_start(ssl[0:1, :], sh_c)
                nc.gpsimd.dma_start(ssl[1:2, :], sl_c)
                nc.scalar.dma_start(ttl2[0:1, :], th_c)
                nc.gpsimd.dma_start(ttl2[1:2, :], tl_c)

            # ---------------- phase 5: fold scale into W^T ----------------
            w0p = res.tile([P, F], bf16)
            w1p = res.tile([P, F], bf16)
            with tc.tile_pool(name="pro3", bufs=2, space="PSUM") as pp3:
                for fc in range(FC):
                    fsl = ts(fc, FCW)
                    pb = pp3.tile([P, FCW], fp32, name="pb")
                    nc.tensor.matmul(pb, ones2, ssl[:, fsl], start=True, stop=True)
                    nc.vector.tensor_tensor(w0p[:, fsl], w0r[:, fsl], pb, Alu.mult)
                    nc.vector.tensor_tensor(w1p[:, fsl], w1r[:, fsl], pb, Alu.mult)
            pro.release()

            # ---------------- main loop over 16 row-tiles ----------------
            with tc.tile_pool(name="mx", bufs=8, space="PSUM") as mxp, \
                 tc.tile_pool(name="zb", bufs=3) as zb, \
                 tc.tile_pool(name="mb", bufs=3) as mb, \
                 tc.tile_pool(name="ub", bufs=3) as ub, \
                 tc.tile_pool(name="nb", bufs=3) as nb, \
                 tc.tile_pool(name="rsb", bufs=6) as rsb:
                mts = {}

                def epilogue(rt):
                    # ut = GAMMA - m (bf16 4x), nt = ut * ps; emitted one tile
                    # late so the DVE never stalls waiting on ACT's relu.
                    mt = mts.pop(rt)
                    ut = ub.tile([P, F], bf16, name="ut")
                    if UTACT and rt % UTACT == 0:
                        nc.scalar.activation(ut, mt, Act.Copy, bias=GAMMA, scale=-1.0)
                    else:
                        nc.vector.tensor_scalar(ut, mt, -1.0, GAMMA, Alu.mult, Alu.add)
                    nt = nb.tile([P, F], bf16, name="nt")
                    if rt in NT_GPS_SET:
                        nc.gpsimd.tensor_tensor(nt, ut, psts[rt], Alu.mult)
                        nc.gpsimd.dma_start(nps_t[rt], nt)
                    else:
                        nc.vector.tensor_tensor(nt, ut, psts[rt], Alu.mult)
                        nc.scalar.dma_start(nps_t[rt], nt)

                for rt in range(RT):
                    rsl = ts(rt, P)
                    pst = psts[rt]
                    px = mxp.tile([P, F], fp32, name="px", tag="px", bufs=2)
                    # pass-type-major: each lhsT loads once, streams 4 chunks
                    ptypes = [(ah0[:, rsl], w0p), (ah1[:, rsl], w1p),
                              (ones2, ttl2)]
                    for pi, (lhsT, rhs) in enumerate(ptypes):
                        for fc in range(FC):
                            nc.tensor.matmul(px[:, ts(fc, FCW)], lhsT, rhs[:, ts(fc, FCW)],
                                             start=(pi == 0), stop=(pi == len(ptypes) - 1))
                    # z = xn * ps over the row-tile; rs = rowsum(z)
                    zt = zb.tile([P, F], fp32, name="zt")
                    rs = rsb.tile([P, 1], fp32, name="rs")
                    nc.vector.scalar_tensor_tensor(
                        zt, px, 1.0, pst, Alu.mult, Alu.mult, accum_out=rs,
                    )
                    # tau = (sum(z)+1)/2047 = (rs+1)/2047
                    ntau = rsb.tile([P, 1], fp32, name="ntau")      # -tau
                    nc.vector.tensor_scalar(ntau, rs, -INV_D1, -INV_D1, Alu.mult, Alu.add)
                    # m = relu(z - tau) = relu(z + ntau)
                    mt = mb.tile([P, F], bf16, name="mt")
                    nc.scalar.activation(mt, zt, Act.Relu, bias=ntau, scale=1.0)
                    nc.scalar.dma_start(m_t[rt], mt)
                    mts[rt] = mt
                    if rt > 0:
                        epilogue(rt - 1)
                epilogue(RT - 1)
            psb.release()

    nc.compile()
    return nc


def _get_nc():
    if "nc" not in _CACHE:
        _CACHE["nc"] = _build_bass()
    return _CACHE["nc"]


def _make_in_maps(a, ps, W, b, bn_w, bn_b):
    import ml_dtypes

    bf = ml_dtypes.bfloat16
    ah = np.ascontiguousarray(a, dtype=np.float32).astype(bf)
    ps16 = np.ascontiguousarray(ps, dtype=np.float32).astype(bf)
    whT_np = np.ascontiguousarray(W.astype(np.float32).T.astype(bf))
    bnw16 = np.ascontiguousarray(bn_w.astype(np.float32).reshape(P, FP))
    bnb16 = np.ascontiguousarray(bn_b.astype(np.float32).reshape(P, FP))
    in_maps = []
    ah_aug = np.ascontiguousarray(
        np.concatenate([ah, np.ones((N, 1), bf)], axis=1))
    for c in range(NCORES):
        rows = slice(c * NSH, (c + 1) * NSH)
        ah_c = ah[rows]
        in_maps.append({
            "ah_aug": ah_aug if KSTATS == "full"
            else np.ascontiguousarray(ah_aug[rows]),
            "ahT": np.ascontiguousarray(ah_c.T),
            "whT": whT_np,
            "ps_in": np.ascontiguousarray(ps16[rows]),
            "bnw16": bnw16,
            "bnb16": bnb16,
        })
    return in_maps


def _maybe_patch_ldwopt():
    """Optionally flip walrus's --enable-ldw-opt (default false in bass_utils)."""
    if os.environ.get("BASS_LDW_OPT") != "1":
        return
    from concourse import bass_utils as bu
    if getattr(bu, "_ldwopt_patched", False):
        return
    orig = bu.run_command

    def patched(argv, **kw):
        argv = [x.replace("--enable-ldw-opt=false", "--enable-ldw-opt=true")
                if isinstance(x, str) else x for x in argv]
        return orig(argv, **kw)

    bu.run_command = patched
    bu._ldwopt_patched = True


def run(a, ps, W, b, bn_w, bn_b, trace=False, **kw):
    """Run the kernel on the 8 NeuronCores; returns ((m, new_ps), BassKernelResults)."""
    from concourse import bass_utils

    _maybe_patch_ldwopt()
    nc = _get_nc()
    in_maps = _make_in_maps(a, ps, W, b, bn_w, bn_b)
    res = bass_utils.run_bass_kernel_spmd(
        nc, in_maps, core_ids=list(range(NCORES)), trace=trace, **kw,
    )
    m = np.concatenate(
        [np.asarray(r["m_out"]).astype(np.float32) for r in res.results], axis=0)
    nps = np.concatenate(
        [np.asarray(r["nps_out"]).astype(np.float32) for r in res.results], axis=0)
    return (m, nps), res


def kernel(a, ps, W, b, bn_w, bn_b):
    (m, nps), _ = run(a, ps, W, b, bn_w, bn_b, trace=False)
    return m, nps


if __name__ == "__main__":
    rng = np.random.default_rng(0)
    a = rng.standard_normal((N, NA), dtype=np.float32)
    ps = rng.random((N, F), dtype=np.float32)
    lim = 1.0 / np.sqrt(NA)
    W = rng.uniform(-lim, lim, (F, NA)).astype(np.float32)
    b = rng.uniform(-lim, lim, (F,)).astype(np.float32)
    bn_w = np.ones((F,), np.float32)
    bn_b = np.zeros((F,), np.float32)
    (m, nps), res = run(a, ps, W, b, bn_w, bn_b)
    print("m", m.shape, m.dtype, "nps", nps.shape)
    print("exec_time_ns:", res.exec_time_ns)
